# revision 11
# baseline (speedup 1.0000x reference)
"""Trainium2 Bass kernel for CenterAlignment (segment-reduce + EMA + normalize + loss).

Contract: kernel(**inputs) takes FULL unsharded numpy inputs
  x:          [65536, 1024] f32
  center_img: [1000, 1024]  f32
  center_skt: [1000, 1024]  f32
  l:          [32768]       int64
and returns the full scalar loss (f32, shape ()).

Strategy (8 NeuronCores, SPMD):
  - Data-parallel shard of x / labels over the sample axis. Crop pairs
    (sample i and i+32768 share label l[i]) are pre-added on-chip.
  - Host-side sharding prep stores each core's x slice in a column-blocked
    bf16 layout [n_batches, n_passes, 128, 2*batch, Q] so every DMA reads
    long contiguous runs per partition (the kernel's internal compute
    precision is bf16; final loss rel err ~1e-6).
  - Per-class partial sums via one-hot matmul: for each 128-sample tile, a
    [128, 1024(padded classes)] bf16 one-hot is built on the vector engine
    (f16 iota vs label), then onehot^T @ xsum accumulates in PSUM across
    4 passes of 256 feature columns; pass 0 carries a constant-2.0 column
    so per-class counts fall out of the same matmuls.
  - All pass results drain into ONE contiguous [1024, 1025] bf16 DRAM
    buffer; a single fused ReduceScatter (instead of one per pass) gives
    each core the global sums for its 128 classes — collective cost here
    is dominated by per-op sync, not bytes.
  - Tail uses factored algebra: per class, with upd = 0.9*ci + 0.1*mean
    and rn = 1/||upd||, the masked loss term ||rn*upd - cs||^2 expands to
    rn^2*A - 2*rn*B + C with A = sum(upd^2), B = sum(upd*cs),
    C = sum(cs^2); A and B further expand into dot products of the global
    sums s with ci/cs/itself (R = sum(ci*s), U = sum(cs*s), S = sum(s^2))
    plus precomputable P = sum(ci^2), T = sum(ci*cs).  The wide tail work
    is then plain mults + row-accumulates on the otherwise-idle Pool/ACT
    engines; everything after is [128,1]-wide.  No full-width
    normalize/diff pass, and the vector engine stays free.
  - A tiny AllGather combines [loss_sum, n_present]; every core computes
    the final scalar.

build_program(K>1) emits the body K times (tile tags reused so iterations
serialize through the Tile dependency graph) — used by test.py to measure
per-iteration hardware time with the dispatch overhead amortized.
"""

import sys

for _p in ("/opt/trn_rl_repo",):
    if _p not in sys.path:
        sys.path.insert(0, _p)

import numpy as np

from concourse import bacc, bass, tile
from concourse import mybir
from concourse import bass_utils

f32 = mybir.dt.float32
f16 = mybir.dt.float16
bf16 = mybir.dt.bfloat16
i32 = mybir.dt.int32

N_CORES = 8
B = 32768              # labels per batch
NUM_CROPS = 2
FEA = 1024             # feature dim
C_PAD = 1024           # classes padded 1000 -> 1024 (8 chunks of 128)
N_CLASSES = 1000
Q = 256                # feature pass width
N_CHUNKS = C_PAD // 128
MOMENTUM = 0.9


def _emit_iter(nc, pools, x01_d, lab_d, ci_d, cs_d, loss_d, groups,
               n_tiles, batch, n_batches, it=0, tail2=True):
    const_pool, oh_pool, x01_pool, xs_pool, qst_pool, psum_pool, dram_pool = pools
    n_passes = FEA // Q
    TOT = FEA + 1  # fused ReduceScatter payload width per class row

    iota_t = const_pool.tile([128, C_PAD], f16, tag="iota")
    nc.gpsimd.iota(iota_t[:], pattern=[[1, C_PAD]], base=0,
                   channel_multiplier=0, allow_small_or_imprecise_dtypes=True)
    ones_t = const_pool.tile([128, 1], f32, tag="ones")
    nc.vector.memset(ones_t[:], 1.0)

    lab_sb = const_pool.tile([128, n_tiles], i32, tag="lab32")
    nc.gpsimd.dma_start(lab_sb[:], lab_d[:].rearrange("(t p) -> p t", p=128))
    labf = const_pool.tile([128, n_tiles], f32, tag="labf")
    nc.vector.tensor_copy(labf[:], lab_sb[:])

    ci_sb = const_pool.tile([128, FEA], f32, tag="ci")
    nc.gpsimd.dma_start(ci_sb[:], ci_d[:, :])
    cs_sb = const_pool.tile([128, FEA], f32, tag="cs")
    nc.gpsimd.dma_start(cs_sb[:], cs_d[:, :])

    # C = sum(cs^2) per class (ACT engine is idle here)
    csq = const_pool.tile([128, 1], f32, tag="csq")
    tmpc = const_pool.tile([128, FEA], f32, tag="tailC")
    nc.scalar.activation(tmpc[:], cs_sb[:], mybir.ActivationFunctionType.Square,
                         accum_out=csq[:])
    if tail2:
        # P = sum(ci^2), T = sum(ci*cs) per class, also precomputable
        Pv = const_pool.tile([128, 1], f32, tag="Pv")
        nc.scalar.activation(tmpc[:], ci_sb[:],
                             mybir.ActivationFunctionType.Square,
                             accum_out=Pv[:])
        Tv = const_pool.tile([128, 1], f32, tag="Tv")
        tmct = const_pool.tile([128, FEA], f32, tag="tailC")
        nc.gpsimd.tensor_tensor(tmct[:], ci_sb[:], cs_sb[:],
                                op=mybir.AluOpType.mult)
        tmct2 = const_pool.tile([128, FEA], f32, tag="tailC2")
        nc.scalar.activation(tmct2[:], tmct[:],
                             mybir.ActivationFunctionType.Identity,
                             accum_out=Tv[:])

    qb_all = dram_pool.tile([C_PAD, TOT], bf16, tag=f"qba{it}", name=f"qba{it}")
    rs_all = dram_pool.tile([C_PAD // N_CORES, TOT], bf16, tag=f"rsa{it}",
                            name=f"rsa{it}")
    ag_in = dram_pool.tile([1, 2], f32, tag=f"agi{it}", name=f"agi{it}")
    ag_out = dram_pool.tile([N_CORES, 2], f32, tag=f"ago{it}", name=f"ago{it}")

    ohs = [None] * n_tiles

    for q in range(n_passes):
        w = Q + 1 if q == 0 else Q   # pass 0 carries the counts column
        coff = 0 if q == 0 else Q * q + 1
        accs = [psum_pool.tile([128, w], f32, tag=f"acc{c}",
                               name=f"acc{c}_{it}_{q}")
                for c in range(N_CHUNKS)]
        for b in range(n_batches):
            x01b = x01_pool.tile([128, NUM_CROPS * batch, Q], bf16, tag="x01b")
            nc.sync.dma_start(x01b[:], x01_d[b, q])
            xsb = xs_pool.tile([128, batch, w], bf16, tag="xsb")
            nc.vector.tensor_tensor(
                xsb[:, :, 0:Q], x01b[:, 0:batch, :],
                x01b[:, batch:2 * batch, :], op=mybir.AluOpType.add)
            if q == 0:
                nc.vector.memset(xsb[:, :, Q:Q + 1], 2.0)
                for j in range(batch):
                    t = b * batch + j
                    oh_t = oh_pool.tile([128, C_PAD], bf16, tag=f"oh{t}",
                                        name=f"oh{t}_{it}")
                    nc.vector.tensor_scalar(
                        oh_t[:], iota_t[:], labf[:, t:t + 1], None,
                        op0=mybir.AluOpType.is_equal)
                    ohs[t] = oh_t
            for j in range(batch):
                t = b * batch + j
                for c in range(N_CHUNKS):
                    nc.tensor.matmul(
                        accs[c][:], ohs[t][:, bass.ts(c, 128)], xsb[:, j, :],
                        start=(t == 0), stop=(t == n_tiles - 1))
        qstage = qst_pool.tile([128, N_CHUNKS, w], bf16, tag="qstage")
        for c in range(N_CHUNKS):
            nc.scalar.copy(qstage[:, c, :], accs[c][:])
        nc.gpsimd.dma_start(
            qb_all[:, coff:coff + w].rearrange("(c p) f -> p c f", p=128),
            qstage[:])

    nc.gpsimd.collective_compute(
        "ReduceScatter", mybir.AluOpType.add, replica_groups=groups,
        ins=[qb_all[:].opt()], outs=[rs_all[:].opt()])

    # ---- factored tail on this core's 128 classes ----
    ms_all = const_pool.tile([128, TOT], bf16, tag="msall")
    nc.gpsimd.dma_start(ms_all[:], rs_all[:, :])
    mcnt = const_pool.tile([128, 1], f32, tag="mcnt")
    nc.vector.tensor_copy(mcnt[:], ms_all[:, Q:Q + 1])
    cnt1 = const_pool.tile([128, 1], f32, tag="cnt1")
    nc.vector.tensor_scalar_max(cnt1[:], mcnt[:], 1.0)
    rec = const_pool.tile([128, 1], f32, tag="rec")
    nc.vector.reciprocal(rec[:], cnt1[:])
    pres = const_pool.tile([128, 1], f32, tag="pres")
    nc.vector.tensor_scalar_min(pres[:], mcnt[:], 1.0)

    if not tail2:
        ssA = const_pool.tile([128, n_passes], f32, tag="ssA")
        ssB = const_pool.tile([128, n_passes], f32, tag="ssB")
        for q in range(n_passes):
            coff = 0 if q == 0 else Q * q + 1
            cols = bass.ts(q, Q)
            msc = const_pool.tile([128, Q], f32, tag="tailA")
            nc.vector.tensor_scalar(msc[:], ms_all[:, coff:coff + Q], rec[:],
                                    1.0 - MOMENTUM, op0=mybir.AluOpType.mult,
                                    op1=mybir.AluOpType.mult)
            upd = const_pool.tile([128, Q], f32, tag="tailB")
            nc.vector.scalar_tensor_tensor(upd[:], in0=ci_sb[:, cols],
                                           scalar=MOMENTUM, in1=msc[:],
                                           op0=mybir.AluOpType.mult,
                                           op1=mybir.AluOpType.add)
            tmpA = const_pool.tile([128, Q], f32, tag="tailD")
            nc.scalar.activation(tmpA[:], upd[:],
                                 mybir.ActivationFunctionType.Square,
                                 accum_out=ssA[:, q:q + 1])
            bt = const_pool.tile([128, Q], f32, tag="tailA")
            nc.vector.tensor_tensor(bt[:], upd[:], cs_sb[:, cols],
                                    op=mybir.AluOpType.mult)
            tmpB = const_pool.tile([128, Q], f32, tag="tailB")
            nc.scalar.activation(tmpB[:], bt[:],
                                 mybir.ActivationFunctionType.Identity,
                                 accum_out=ssB[:, q:q + 1])

        A = const_pool.tile([128, 1], f32, tag="A")
        t4 = const_pool.tile([128, n_passes], f32, tag="t4")
        nc.scalar.activation(t4[:], ssA[:],
                             mybir.ActivationFunctionType.Identity,
                             accum_out=A[:])
        Bv = const_pool.tile([128, 1], f32, tag="Bv")
        t5 = const_pool.tile([128, n_passes], f32, tag="t5")
        nc.scalar.activation(t5[:], ssB[:],
                             mybir.ActivationFunctionType.Identity,
                             accum_out=Bv[:])
    else:
        # dot products of the global sums s with ci / cs / itself; the EMA
        # folds into scalars: with c = max(counts,1),
        #   A = sum(upd^2)   = 0.81*P + 0.18*R/c + 0.01*S/c^2
        #   B = sum(upd*cs)  = 0.90*T + 0.10*U/c
        # where R = sum(ci*s), U = sum(cs*s), S = sum(s*s).  Wide ops are
        # plain mults on the (idle) Pool engine + ACT accumulates — the
        # vector engine stays free for the next iteration's main loop.
        ssR = const_pool.tile([128, n_passes], f32, tag="ssR")
        ssU = const_pool.tile([128, n_passes], f32, tag="ssU")
        ssS = const_pool.tile([128, n_passes], f32, tag="ssS")
        for q in range(n_passes):
            coff = 0 if q == 0 else Q * q + 1
            cols = bass.ts(q, Q)
            sq = ms_all[:, coff:coff + Q]
            rt = const_pool.tile([128, Q], f32, tag="tailA")
            nc.gpsimd.tensor_tensor(rt[:], ci_sb[:, cols], sq,
                                    op=mybir.AluOpType.mult)
            rt2 = const_pool.tile([128, Q], f32, tag="tailB")
            nc.scalar.activation(rt2[:], rt[:],
                                 mybir.ActivationFunctionType.Identity,
                                 accum_out=ssR[:, q:q + 1])
            ut = const_pool.tile([128, Q], f32, tag="tailA")
            nc.gpsimd.tensor_tensor(ut[:], cs_sb[:, cols], sq,
                                    op=mybir.AluOpType.mult)
            ut2 = const_pool.tile([128, Q], f32, tag="tailB")
            nc.scalar.activation(ut2[:], ut[:],
                                 mybir.ActivationFunctionType.Identity,
                                 accum_out=ssU[:, q:q + 1])
            st2 = const_pool.tile([128, Q], f32, tag="tailD")
            nc.scalar.activation(st2[:], sq,
                                 mybir.ActivationFunctionType.Square,
                                 accum_out=ssS[:, q:q + 1])

        Rv = const_pool.tile([128, 1], f32, tag="Rv")
        t4 = const_pool.tile([128, n_passes], f32, tag="t4")
        nc.scalar.activation(t4[:], ssR[:],
                             mybir.ActivationFunctionType.Identity,
                             accum_out=Rv[:])
        Uv = const_pool.tile([128, 1], f32, tag="Uv")
        t5 = const_pool.tile([128, n_passes], f32, tag="t5")
        nc.scalar.activation(t5[:], ssU[:],
                             mybir.ActivationFunctionType.Identity,
                             accum_out=Uv[:])
        Sv = const_pool.tile([128, 1], f32, tag="Sv")
        t5b = const_pool.tile([128, n_passes], f32, tag="t5b")
        nc.scalar.activation(t5b[:], ssS[:],
                             mybir.ActivationFunctionType.Identity,
                             accum_out=Sv[:])

        rc2 = const_pool.tile([128, 1], f32, tag="rc2")
        nc.vector.tensor_tensor(rc2[:], rec[:], rec[:],
                                op=mybir.AluOpType.mult)
        aw1 = const_pool.tile([128, 1], f32, tag="aw1")
        nc.vector.tensor_tensor(aw1[:], Rv[:], rec[:],
                                op=mybir.AluOpType.mult)
        aw2 = const_pool.tile([128, 1], f32, tag="aw2")
        nc.vector.tensor_tensor(aw2[:], Sv[:], rc2[:],
                                op=mybir.AluOpType.mult)
        v1 = const_pool.tile([128, 1], f32, tag="v1")
        nc.vector.tensor_scalar(v1[:], aw1[:], 2 * MOMENTUM * (1 - MOMENTUM),
                                None, op0=mybir.AluOpType.mult)
        v2 = const_pool.tile([128, 1], f32, tag="v2")
        nc.vector.scalar_tensor_tensor(v2[:], in0=Pv[:],
                                       scalar=MOMENTUM * MOMENTUM, in1=v1[:],
                                       op0=mybir.AluOpType.mult,
                                       op1=mybir.AluOpType.add)
        A = const_pool.tile([128, 1], f32, tag="A")
        nc.vector.scalar_tensor_tensor(
            A[:], in0=aw2[:], scalar=(1 - MOMENTUM) * (1 - MOMENTUM),
            in1=v2[:], op0=mybir.AluOpType.mult, op1=mybir.AluOpType.add)
        bw = const_pool.tile([128, 1], f32, tag="bw")
        nc.vector.tensor_tensor(bw[:], Uv[:], rec[:],
                                op=mybir.AluOpType.mult)
        b1 = const_pool.tile([128, 1], f32, tag="b1")
        nc.vector.tensor_scalar(b1[:], bw[:], 1 - MOMENTUM, None,
                                op0=mybir.AluOpType.mult)
        Bv = const_pool.tile([128, 1], f32, tag="Bv")
        nc.vector.scalar_tensor_tensor(Bv[:], in0=Tv[:], scalar=MOMENTUM,
                                       in1=b1[:], op0=mybir.AluOpType.mult,
                                       op1=mybir.AluOpType.add)
    ssg = const_pool.tile([128, 1], f32, tag="ssg")
    nc.vector.tensor_scalar_max(ssg[:], A[:], 1e-30)
    ssr = const_pool.tile([128, 1], f32, tag="ssr")
    nc.vector.reciprocal(ssr[:], ssg[:])
    rn = const_pool.tile([128, 1], f32, tag="rn")
    nc.scalar.activation(rn[:], ssr[:], mybir.ActivationFunctionType.Sqrt)
    r2 = const_pool.tile([128, 1], f32, tag="r2")
    nc.vector.tensor_tensor(r2[:], rn[:], rn[:], op=mybir.AluOpType.mult)
    t1v = const_pool.tile([128, 1], f32, tag="t1v")
    nc.vector.tensor_tensor(t1v[:], r2[:], A[:], op=mybir.AluOpType.mult)
    t2v = const_pool.tile([128, 1], f32, tag="t2v")
    nc.vector.tensor_tensor(t2v[:], rn[:], Bv[:], op=mybir.AluOpType.mult)
    t6 = const_pool.tile([128, 1], f32, tag="t6")
    nc.vector.scalar_tensor_tensor(t6[:], in0=t2v[:], scalar=-2.0, in1=t1v[:],
                                   op0=mybir.AluOpType.mult,
                                   op1=mybir.AluOpType.add)
    pc = const_pool.tile([128, 1], f32, tag="pc")
    nc.vector.tensor_tensor(pc[:], t6[:], csq[:], op=mybir.AluOpType.add)
    stack = const_pool.tile([128, 2], f32, tag="stack")
    nc.vector.tensor_tensor(stack[:, 0:1], pc[:], pres[:],
                            op=mybir.AluOpType.mult)
    nc.vector.tensor_copy(stack[:, 1:2], pres[:])

    red_ps = psum_pool.tile([1, 2], f32, tag="acc0", name=f"redps{it}")
    nc.tensor.matmul(red_ps[:], ones_t[:], stack[:], start=True, stop=True)
    red_sb = const_pool.tile([1, 2], f32, tag="redsb")
    nc.scalar.copy(red_sb[:], red_ps[:])
    nc.gpsimd.dma_start(ag_in[:, :], red_sb[:])
    nc.gpsimd.collective_compute(
        "AllGather", mybir.AluOpType.bypass, replica_groups=groups,
        ins=[ag_in[:].opt()], outs=[ag_out[:].opt()])
    ag_sb = const_pool.tile([1, N_CORES * 2], f32, tag="agsb")
    nc.gpsimd.dma_start(
        ag_sb[:],
        ag_out[:, :].rearrange("r c -> (r c)").rearrange("(p f) -> p f", p=1))
    f8 = const_pool.tile([1, 8], f32, tag="f8")
    nc.vector.tensor_tensor(f8[:], ag_sb[:, 0:8], ag_sb[:, 8:16],
                            op=mybir.AluOpType.add)
    f4 = const_pool.tile([1, 4], f32, tag="f4")
    nc.vector.tensor_tensor(f4[:], f8[:, 0:4], f8[:, 4:8],
                            op=mybir.AluOpType.add)
    fin = const_pool.tile([1, 2], f32, tag="fin")
    nc.vector.tensor_tensor(fin[:], f4[:, 0:2], f4[:, 2:4],
                            op=mybir.AluOpType.add)
    finv = const_pool.tile([1, 1], f32, tag="finv")
    nc.vector.reciprocal(finv[:], fin[:, 1:2])
    lsb = const_pool.tile([1, 1], f32, tag="lsb")
    nc.vector.tensor_tensor(lsb[:], fin[:, 0:1], finv[:],
                            op=mybir.AluOpType.mult)
    nc.gpsimd.dma_start(loss_d[:].rearrange("(p o) -> p o", o=1), lsb[:])


def build_program(rows_per_core: int = B // N_CORES, batch: int = 8,
                  K: int = 1, tail2: bool = True):
    """Build the SPMD Bass program (same graph on all 8 cores).

    rows_per_core: number of crop-PAIRS this core owns (default 4096).
    K: number of body repetitions (K>1 only for steady-state timing).
    """
    assert rows_per_core % 128 == 0
    n_tiles = rows_per_core // 128
    batch = min(batch, n_tiles)
    assert n_tiles % batch == 0
    n_batches = n_tiles // batch
    n_passes = FEA // Q

    nc = bacc.Bacc(
        "TRN2",
        target_bir_lowering=False,
        debug=False,
        enable_asserts=False,
        num_devices=N_CORES,
    )

    x01_d = nc.dram_tensor(
        "x01", [n_batches, n_passes, 128, NUM_CROPS * batch, Q], bf16,
        kind="ExternalInput",
    )
    lab_d = nc.dram_tensor("labels", [rows_per_core], i32, kind="ExternalInput")
    ci_d = nc.dram_tensor("ci", [128, FEA], f32, kind="ExternalInput")
    cs_d = nc.dram_tensor("cs", [128, FEA], f32, kind="ExternalInput")
    loss_d = nc.dram_tensor("loss", [1], f32, kind="ExternalOutput")

    groups = [list(range(N_CORES))]

    with tile.TileContext(nc) as tc:
        with (
            tc.tile_pool(name="const", bufs=1) as const_pool,
            tc.tile_pool(name="oh", bufs=1) as oh_pool,
            tc.tile_pool(name="x01p", bufs=2) as x01_pool,
            tc.tile_pool(name="xsp", bufs=2) as xs_pool,
            tc.tile_pool(name="qst", bufs=2) as qst_pool,
            tc.tile_pool(name="psum", bufs=1, space="PSUM") as psum_pool,
            tc.tile_pool(name="dram", bufs=1, space="DRAM") as dram_pool,
        ):
            pools = (const_pool, oh_pool, x01_pool, xs_pool, qst_pool,
                     psum_pool, dram_pool)
            for it in range(K):
                _emit_iter(nc, pools, x01_d, lab_d, ci_d, cs_d, loss_d,
                           groups, n_tiles, batch, n_batches, it=it,
                           tail2=tail2)

    nc.compile()
    return nc


def _fast_bf16(a):
    """Vectorized f32 -> bf16 cast (round-to-nearest-even); ml_dtypes'
    astype is a scalar loop and takes minutes on 256 MB."""
    import ml_dtypes

    u = np.ascontiguousarray(a, dtype=np.float32).view(np.uint32)
    rounded = u + 0x7FFF + ((u >> 16) & 1)
    return (rounded >> 16).astype(np.uint16).view(ml_dtypes.bfloat16)


def make_in_maps(x, center_img, center_skt, l, rows_per_core=None, batch=8):
    """Shard full inputs into per-core input maps (column-blocked bf16 x)."""

    n = x.shape[0] // NUM_CROPS
    if rows_per_core is None:
        rows_per_core = n // N_CORES
    x = np.ascontiguousarray(x, dtype=np.float32)
    l = np.ascontiguousarray(l).astype(np.int32)
    n_tiles = rows_per_core // 128
    batch = min(batch, n_tiles)
    n_batches = n_tiles // batch
    n_passes = FEA // Q
    ci_pad = np.zeros((C_PAD, FEA), np.float32)
    ci_pad[: center_img.shape[0]] = center_img
    cs_pad = np.zeros((C_PAD, FEA), np.float32)
    cs_pad[: center_skt.shape[0]] = center_skt
    in_maps = []
    for k in range(N_CORES):
        r0 = k * rows_per_core
        r1 = r0 + rows_per_core
        # [2, n_batches, batch, 128, n_passes, Q] -> [nb, n_passes, 128, 2*batch, Q]
        xs = np.stack([x[r0:r1], x[n + r0: n + r1]])
        xs = xs.reshape(2, n_batches, batch, 128, n_passes, Q)
        xs = xs.transpose(1, 4, 3, 0, 2, 5)
        xs = xs.reshape(n_batches, n_passes, 128, NUM_CROPS * batch, Q)
        in_maps.append(
            {
                "x01": _fast_bf16(np.ascontiguousarray(xs)),
                "labels": np.ascontiguousarray(l[r0:r1]),
                "ci": np.ascontiguousarray(ci_pad[k * 128: (k + 1) * 128]),
                "cs": np.ascontiguousarray(cs_pad[k * 128: (k + 1) * 128]),
            }
        )
    return in_maps


_CACHED_NC = None


def _get_nc():
    global _CACHED_NC
    if _CACHED_NC is None:
        _CACHED_NC = build_program()
    return _CACHED_NC


def kernel(x, center_img, center_skt, l):
    nc = _get_nc()
    in_maps = make_in_maps(x, center_img, center_skt, l)
    res = bass_utils.run_bass_kernel_spmd(nc, in_maps, core_ids=list(range(N_CORES)))
    loss = np.asarray(res.results[0]["loss"], dtype=np.float32)
    return loss.reshape(()).astype(np.float32)
